# revision 64
# baseline (speedup 1.0000x reference)
import os, sys
import numpy as np

sys.path.insert(0, "/opt/trn_rl_repo")

from concourse import bass, bacc, bass_utils
from concourse import mybir
from concourse.tile import TileContext

F32 = mybir.dt.float32
F16 = mybir.dt.float16
ALU = mybir.AluOpType
ACTF = mybir.ActivationFunctionType
AX = mybir.AxisListType

A = 32          # in_maps
B = 32          # out_maps
C = 16          # atoms
H = 64
W = 64
NCORES = 8
ROWS = H // NCORES
NPOS = ROWS * W             # 512 positions per core
NCHUNK = 128
NCH = NPOS // NCHUNK        # 4 chunks
CB = C * B                  # 512 (free order is (b, c))
EPS = 1e-4
G = 14                      # a-values per packed partition group
NGRP = 3                    # ceil(A/G)
NP_PK = 9 * G               # 126 partitions for packed p1 operands
HB = 16                     # b-half size
HBC = HB * C                # 256

# engine split: how many of the 16 b's per half go to DVE for the delta mult
DM_DVE = 13

_CACHE = {}


def _build_nc(num_routes: int):
    nc = bacc.Bacc(None, target_bir_lowering=False)

    # stacked patches px rows: [pl, ph, ph]; weights wx rows: [wh, wl, wh]
    # one 27-row matmul = pl*wh + ph*wl + ph*wh (the 3-term f16 product)
    px_d = nc.declare_dram_parameter("px", [27, A, NPOS], F16, isOutput=False)
    wx_d = nc.declare_dram_parameter("wx", [27, A, CB], F16, isOutput=False)
    out_d = nc.declare_dram_parameter("out", [NPOS, B, C], F32, isOutput=True)

    with TileContext(nc) as tc:
        with (
            tc.tile_pool(name="wconst", bufs=1) as wc,
            tc.tile_pool(name="pchp", bufs=2) as pchp,
            tc.tile_pool(name="psv", bufs=5, space="PSUM") as psv,
            tc.tile_pool(name="psp1", bufs=2, space="PSUM") as psp1,
            tc.tile_pool(name="vp", bufs=1) as vpool,
            tc.tile_pool(name="tp", bufs=1) as tpool,
            tc.tile_pool(name="smp", bufs=1) as sm,
        ):
            # ---- hoisted weights (loaded once) ----
            # 27 stacked rows pair with px so one matmul does all 3 f16
            # product terms: [pl,ph,ph] x [wh,wl,wh]
            wx = wc.tile([27, A * CB], F16, tag="wx")
            wx3 = wx[:].rearrange("q (a n) -> q a n", a=A)
            for sl in range(4):
                asl = slice(sl * 8, (sl + 1) * 8)
                nc.scalar.dma_start(out=wx3[:, asl, :],
                                    in_=wx_d.ap()[:, asl, :])

            for k in range(NCH):
                ksl = slice(k * NCHUNK, (k + 1) * NCHUNK)

                # ---- patch DMAs for this chunk ----
                px = pchp.tile([27, A * NCHUNK], F16, tag="px")
                px3 = px[:].rearrange("q (a n) -> q a n", a=A)
                for sl in range(2):
                    asl = slice(sl * 16, (sl + 1) * 16)
                    nc.sync.dma_start(out=px3[:, asl, :],
                                      in_=px_d.ap()[:, asl, ksl])
                # ---- votes + p1 matmuls + PSUM->SBUF copies ----
                p1ps = psp1.tile([NCHUNK, CB], F32)
                # v32: two half tiles, a-major [p, (a32 b16 c16)] f32 so a
                # PSUM bank (2 a-values) copies out contiguously
                v32 = [vpool.tile([NCHUNK, A * (B // 2) * C], F32,
                                  tag=f"v32_{h}", name=f"v32_{h}")
                       for h in range(2)]
                vh4 = [v32[h][:].rearrange("p (a b c) -> p a b c", a=A,
                                           b=B // 2)
                       for h in range(2)]
                v4 = [vh4[q // 2][:, :, (q % 2) * 8:(q % 2) * 8 + 8, :]
                      .rearrange("p a b c -> p b a c")
                      for q in range(4)]
                HW2 = (B // 2) * C          # 256 cols per half
                for aq in range(A // 4):
                    aa = [4 * aq + j for j in range(4)]
                    for a in aa:
                        nc.tensor.matmul(out=p1ps[:], lhsT=px3[0:27, a, :],
                                         rhs=wx3[0:27, a, :],
                                         start=(a == 0), stop=(a == A - 1))
                    for h in range(2):
                        # 2-bank tile: 4 a-values x one b-half
                        vps = psv.tile([NCHUNK, 2 * CB], F32, tag="vps", bufs=3)
                        csl = slice(h * HW2, (h + 1) * HW2)
                        for j, a in enumerate(aa):
                            nc.tensor.matmul(
                                out=vps[:, j * HW2:(j + 1) * HW2],
                                lhsT=px3[0:27, a, :],
                                rhs=wx3[0:27, a, csl],
                                start=(j % 2 == 0), stop=(j % 2 == 1))
                        dst = v32[h][:].rearrange(
                            "p (a n) -> p a n", a=A // 4)[:, aq, :]
                        act_copy = (aq % 2 == h) if k == 0 else \
                            (h == 0 or aq < 4)
                        if act_copy:
                            nc.scalar.copy(out=dst, in_=vps[:])
                        else:
                            nc.vector.tensor_copy(out=dst, in_=vps[:])

                # ---- routing: 4 independent b-quarter chains, stage-interleaved ----
                # delta_i = fac[b] * sum_c v * praw  (fac factored out of the
                # contraction so the Act sqrt hides behind the big mult)
                NQ = 4
                QB = B // NQ            # 8
                QBC = QB * C            # 128
                tiles = []
                for q in range(NQ):
                    t = {}
                    t["l"] = sm.tile([NCHUNK, QB * A], F32, tag=f"l_{q}",
                                     name=f"l_{q}")
                    t["e32"] = sm.tile([NCHUNK, QB * A], F32, tag=f"e_{q}",
                                       name=f"e_{q}")
                    t["lsm16"] = sm.tile([NCHUNK, QB * A], F16, tag=f"lsm_{q}",
                                         name=f"lsm_{q}")
                    t["praw"] = sm.tile([NCHUNK, QBC], F32, tag=f"praw_{q}",
                                        name=f"praw_{q}")
                    t["out32"] = sm.tile([NCHUNK, QBC], F32, tag=f"out32_{q}",
                                         name=f"out32_{q}")
                    t["p2"] = sm.tile([NCHUNK, QBC], F32, tag=f"p2_{q}",
                                      name=f"p2_{q}")
                    t["treeo"] = sm.tile([NCHUNK, QBC], F32, tag=f"treeo_{q}",
                                         name=f"treeo_{q}")
                    for nm in ("sq", "sqv", "nrm", "f1", "den", "fac"):
                        t[nm] = sm.tile([NCHUNK, QB], F32, tag=f"{nm}_{q}",
                                        name=f"{nm}_{q}")
                    tiles.append(t)

                def fac_pre_ops(q, X3, s):
                    t = tiles[q]
                    p23 = t["p2"][:].rearrange("p (b c) -> p b c", b=QB)
                    nc.scalar.activation(out=p23, in_=X3, func=ACTF.Square)
                    nc.vector.tensor_reduce(out=t["sq"][:], in_=p23,
                                            axis=AX.X, op=ALU.add)
                    nc.vector.tensor_scalar(out=t["sqv"][:], in0=t["sq"][:],
                                            scalar1=s * s, scalar2=EPS,
                                            op0=ALU.mult, op1=ALU.add)
                    nc.scalar.activation(out=t["nrm"][:], in_=t["sqv"][:],
                                         func=ACTF.Sqrt)

                def fac_post_ops(q, s):
                    t = tiles[q]
                    nc.vector.scalar_tensor_tensor(
                        out=t["den"][:], in0=t["sqv"][:], scalar=1.0,
                        in1=t["nrm"][:], op0=ALU.add, op1=ALU.mult)
                    nc.vector.tensor_scalar_add(out=t["den"][:],
                                                in0=t["den"][:], scalar1=EPS)
                    nc.vector.reciprocal(out=t["den"][:], in_=t["den"][:])
                    nc.vector.scalar_tensor_tensor(
                        out=t["fac"][:], in0=t["sqv"][:], scalar=s,
                        in1=t["den"][:], op0=ALU.mult, op1=ALU.mult)

                p1v = p1ps[:].rearrange("p (b c) -> p b c", b=B)

                for q in range(NQ):
                    nc.scalar.copy(out=tiles[q]["praw"][:].rearrange(
                        "p (b c) -> p b c", b=QB),
                        in_=p1v[:, q * QB:(q + 1) * QB, :])

                for it in range(2, num_routes + 1):
                    s = (1.0 / A) if it == 2 else 1.0
                    for q in range(NQ):
                        if it == 2:
                            X3 = p1v[:, q * QB:(q + 1) * QB, :]
                        else:
                            X3 = tiles[q]["treeo"][:].rearrange(
                                "p (b c) -> p b c", b=QB)
                        fac_pre_ops(q, X3, s)

                    srcs = []
                    for q in range(NQ):
                        key = "praw" if it == 2 else "treeo"
                        srcs.append(tiles[q][key][:].rearrange(
                            "p (b c) -> p b c", b=QB))
                    tmps = []
                    for q in range(NQ):
                        tmp32 = tpool.tile([NCHUNK, QB * A * C], F32,
                                           tag=f"tmp32_{q}",
                                           name=f"tmp32_{q}")
                        tmps.append(tmp32)
                    for q in range(NQ):
                        t4 = tmps[q][:].rearrange("p (b a c) -> p b a c",
                                                  b=QB, a=A)
                        dm = 6 if (q % 2 == 0) else 5
                        for (eng, b0, b1) in ((nc.vector, 0, dm),
                                              (nc.gpsimd, dm, QB)):
                            eng.tensor_tensor(
                                out=t4[:, b0:b1, :, :],
                                in0=v4[q][:, b0:b1, :, :],
                                in1=srcs[q][:, b0:b1, :].unsqueeze(2)
                                    .to_broadcast([NCHUNK, b1 - b0, A, C]),
                                op=ALU.mult)
                    for (w0, w1) in ((8, 16), (4, 8), (2, 4)):
                        for q in range(NQ):
                            t4 = tmps[q][:].rearrange(
                                "p (b a c) -> p b a c", b=QB, a=A)
                            nc.gpsimd.tensor_tensor(
                                out=t4[:, :, :, 0:w0],
                                in0=t4[:, :, :, 0:w0],
                                in1=t4[:, :, :, w0:w1],
                                op=ALU.add)
                    for q in range(NQ):
                        t4 = tmps[q][:].rearrange("p (b a c) -> p b a c",
                                                  b=QB, a=A)
                        l3 = tiles[q]["l"][:].rearrange("p (b a) -> p b a",
                                                        b=QB)
                        nc.gpsimd.tensor_tensor(out=l3, in0=t4[:, :, :, 0],
                                                in1=t4[:, :, :, 1],
                                                op=ALU.add)
                    for q in range(NQ):
                        fac_post_ops(q, s)
                    for q in range(NQ):
                        l3 = tiles[q]["l"][:].rearrange("p (b a) -> p b a",
                                                        b=QB)
                        nc.vector.tensor_tensor(
                            out=l3, in0=l3,
                            in1=tiles[q]["fac"][:].unsqueeze(2).to_broadcast(
                                [NCHUNK, QB, A]),
                            op=ALU.mult)
                    if it > 2:
                        for q in range(NQ):
                            nc.gpsimd.tensor_tensor(
                                out=tiles[q]["l"][:], in0=tiles[q]["l"][:],
                                in1=tiles[q]["lsm16"][:], op=ALU.add)

                    for q in range(NQ):
                        nc.scalar.activation(out=tiles[q]["e32"][:],
                                             in_=tiles[q]["l"][:],
                                             func=ACTF.Exp)
                    for q in range(NQ):
                        nc.vector.tensor_reduce(
                            out=tiles[q]["sq"][:],
                            in_=tiles[q]["e32"][:].rearrange(
                                "p (b a) -> p b a", b=QB),
                            axis=AX.X, op=ALU.add)
                    for q in range(NQ):
                        nc.vector.reciprocal(out=tiles[q]["sq"][:],
                                             in_=tiles[q]["sq"][:])
                    for q in range(NQ):
                        eng = nc.vector if q % 2 == 0 else nc.gpsimd
                        eng.tensor_tensor(
                            out=tiles[q]["lsm16"][:].rearrange(
                                "p (b a) -> p b a", b=QB),
                            in0=tiles[q]["e32"][:].rearrange(
                                "p (b a) -> p b a", b=QB),
                            in1=tiles[q]["sq"][:].unsqueeze(2).to_broadcast(
                                [NCHUNK, QB, A]),
                            op=ALU.mult)

                    for q in range(NQ):
                        t16 = tmps[q][:].bitcast(F16)[:, :QB * A * C]\
                            .rearrange("p (b a c) -> p b a c", b=QB, a=A)
                        nc.gpsimd.tensor_tensor(
                            out=t16,
                            in0=v4[q][:],
                            in1=tiles[q]["lsm16"][:].rearrange(
                                "p (b a) -> p b a", b=QB)
                                .unsqueeze(3).to_broadcast(
                                    [NCHUNK, QB, A, C]),
                            op=ALU.mult)
                    for (w0, w1) in ((16, 32), (8, 16), (4, 8), (2, 4)):
                        for q in range(NQ):
                            t16 = tmps[q][:].bitcast(F16)[:, :QB * A * C]\
                                .rearrange("p (b a c) -> p b a c", b=QB, a=A)
                            nc.vector.tensor_tensor(
                                out=t16[:, :, 0:w0, :],
                                in0=t16[:, :, 0:w0, :],
                                in1=t16[:, :, w0:w1, :],
                                op=ALU.add)
                    for q in range(NQ):
                        t16 = tmps[q][:].bitcast(F16)[:, :QB * A * C]\
                            .rearrange("p (b a c) -> p b a c", b=QB, a=A)
                        nc.vector.tensor_tensor(
                            out=tiles[q]["treeo"][:].rearrange(
                                "p (b c) -> p b c", b=QB),
                            in0=t16[:, :, 0, :], in1=t16[:, :, 1, :],
                            op=ALU.add)

                fs = (1.0 / A) if num_routes == 1 else 1.0
                for q in range(NQ):
                    key = "praw" if num_routes == 1 else "treeo"
                    X3 = tiles[q][key][:].rearrange("p (b c) -> p b c", b=QB)
                    fac_pre_ops(q, X3, fs)
                for q in range(NQ):
                    fac_post_ops(q, fs)
                for q in range(NQ):
                    t = tiles[q]
                    xsrc = t["praw"] if num_routes == 1 else t["treeo"]
                    nc.vector.tensor_tensor(
                        out=t["out32"][:].rearrange("p (b c) -> p b c", b=QB),
                        in0=xsrc[:].rearrange("p (b c) -> p b c", b=QB),
                        in1=t["fac"][:].unsqueeze(2).to_broadcast(
                            [NCHUNK, QB, C]),
                        op=ALU.mult)
                for q in range(NQ):
                    nc.sync.dma_start(
                        out=out_d.ap()[ksl, q * QB:(q + 1) * QB, :],
                        in_=tiles[q]["out32"][:].rearrange(
                            "p (b c) -> p b c", b=QB))

    nc.compile()
    return nc


def _prep_inputs(x, weights):
    f16 = np.float16
    xp = np.zeros((A, H + 2, W + 2), dtype=np.float32)
    xp[:, 1:-1, 1:-1] = x
    # weights (3,3,A,C,B) -> (9, A, B, C) -> (9, A, CB)
    wv = np.ascontiguousarray(
        weights.reshape(9, A, C, B).transpose(0, 1, 3, 2)).reshape(9, A, CB)
    wh = wv.astype(f16)
    wl = (wv - wh.astype(np.float32)).astype(f16)
    # stacked weights [27, A, CB]: rows 0-17 = [wh, wl] (cross), 18-26 = wh
    wx = np.concatenate([wh, wl, wh], axis=0)

    per_core = []
    for core in range(NCORES):
        r0 = core * ROWS
        pat = np.empty((9, A, ROWS, W), dtype=np.float32)
        for dp in range(3):
            for dq in range(3):
                pat[dp * 3 + dq] = xp[:, r0 + dp:r0 + dp + ROWS, dq:dq + W]
        patf = np.ascontiguousarray(pat.reshape(9, A, NPOS))
        p_h = patf.astype(f16)
        p_l = (patf - p_h.astype(np.float32)).astype(f16)
        # stacked patches [27, A, NPOS]: rows 0-17 = [pl, ph] (cross), 18-26 = ph
        px = np.concatenate([p_l, p_h, p_h], axis=0)
        per_core.append({"px": px, "wx": wx})
    return per_core


def kernel(x=None, weights=None, num_routes=3, **kw):
    x = np.asarray(x, dtype=np.float32)
    weights = np.asarray(weights, dtype=np.float32)
    nr = int(num_routes)

    if nr not in _CACHE:
        _CACHE[nr] = _build_nc(nr)
    nc = _CACHE[nr]

    in_maps = _prep_inputs(x, weights)
    res = bass_utils.run_bass_kernel_spmd(nc, in_maps, core_ids=list(range(NCORES)))

    out = np.empty((B, C, H, W), dtype=np.float32)
    for core in range(NCORES):
        o = np.asarray(res.results[core]["out"]).reshape(ROWS, W, B, C)
        out[:, :, core * ROWS:(core + 1) * ROWS, :] = o.transpose(2, 3, 0, 1)
    return out


def profile_once(inputs):
    """Run once with NTFF tracing on core 0 and return HW exec time in ns."""
    x = np.asarray(inputs["x"], dtype=np.float32)
    weights = np.asarray(inputs["weights"], dtype=np.float32)
    nr = int(inputs.get("num_routes", 3))
    if nr not in _CACHE:
        _CACHE[nr] = _build_nc(nr)
    nc = _CACHE[nr]
    in_maps = _prep_inputs(x, weights)
    res = bass_utils.run_bass_kernel_spmd(nc, in_maps,
                                          core_ids=list(range(NCORES)),
                                          trace=True, trace_cores=[0])
    if res.exec_time_ns is not None:
        return int(res.exec_time_ns)
    raise RuntimeError("no exec_time_ns from trace")
